# revision 5
# baseline (speedup 1.0000x reference)
"""AffinityLoss on 8 TRN2 NeuronCores (Bass/Tile).

Math: 3x3-unfold affinity loss = mean BCE-with-logits between per-pixel 9x9
channel Gram matrices and label-equality maps. Reformulated over 13 canonical
relative shifts s=(dr,dc) with integer edge-weight profiles wy/wx:

    loss_sum = sum_s mult_s * sum_{y,x} wy_s(y) wx_s(x) * ln(1 + exp(w_s(y,x)))
    w_s = (1 - 2*E_s) * Corr_s,   Corr_s(y,x) = sum_c A[c,y,x] A[c,y+dr,x+dc],
    E_s = [labels equal across the shift]

(BCE-with-logits == softplus((1-2E)*x); max |Corr| ~ 66 so exp is safe.)

Sharding: data-parallel, core k owns image k//2, rows [192*(k%2), +192).
Per core: pass0 = 128 rows full width; pass1 = 64 rows x 2 x-half groups
stacked on partitions.

Shifts are processed in CHAINS grouped by (dr, dc-parity) so that the
add-tree / sign / exp stages run as single wide instructions over G=2-3
shifts (4D tiles), and the work is split across three engines:
  - DVE (bottleneck): bf16 products (c0..17 -> pb, c18 -> pc) and the two
    big tree levels, all in 2x perf mode.
  - GpSimd: low tree levels, label-diff subtract, sign-apply multiply,
    border-column exports (it only supports add/mult/subtract).
  - ScalarE: s=(0,0) products as Square, sh = Sign(Square(d) - 0.5),
    Exp, Ln(bias=1, accum_out) per-row loss sums.
wx border deviations live within 2 cols of the edges; those columns are
exported exactly and corrected on host in f64 together with the wy
weighting and the cross-core reduction.
"""

import os
import sys

import numpy as np

for _p in ("/root/.axon_site", "/root/.axon_site/_ro/trn_rl_repo",
           "/root/.axon_site/_ro/pypackages"):
    if os.path.isdir(_p) and _p not in sys.path:
        sys.path.append(_p)

import ml_dtypes  # noqa: E402

N, C, H, W = 4, 19, 384, 384
K = 3
HP = WP = H - K + 1  # 382
N_CORES = 8
ROWS_PER_CORE = 192
SHIFTS = [(0, 0), (0, 1), (0, 2)] + [(dr, dc) for dr in (1, 2) for dc in (-2, -1, 0, 1, 2)]
NS = len(SHIFTS)  # 13
PASS_GEOM = [
    dict(row0=0, DW=W, TW=W + 8),      # tiles [128, C, 392], data cols 2..385
    dict(row0=128, DW=194, TW=200),    # 64 rows x 2 x-half groups
]
GROUP_X0 = [0, 190]   # pass1 group g covers x in [190g, 190g+194)
COLS = 2 * NS        # one ln-accum column per (pass, shift)
BCOLS = 2 * NS * 4   # 4 exported border cols per (pass, shift)
LGROWS = 196

# chains: shifts sharing (dr, alignment parity); 'sq' = s=(0,0) square path
CHAINS = [
    dict(kind='ev', dr=0, dcs=[2], sis=[2]),
    dict(kind='sq', dr=0, dcs=[0], sis=[0]),
    dict(kind='od', dr=0, dcs=[1], sis=[1]),
    dict(kind='ev', dr=1, dcs=[-2, 0, 2], sis=[3, 5, 7]),
    dict(kind='od', dr=1, dcs=[-1, 1], sis=[4, 6]),
    dict(kind='ev', dr=2, dcs=[-2, 0, 2], sis=[8, 10, 12]),
    dict(kind='od', dr=2, dcs=[-1, 1], sis=[9, 11]),
]
GMAX = 3


def _wx_profile(dc, x):
    w = np.zeros_like(x, dtype=np.float64)
    for ca in range(K):
        if 0 <= ca + dc < K:
            w += ((x - ca >= 0) & (x - ca < WP))
    return w


def _wy_profile(dr, y):
    w = np.zeros_like(y, dtype=np.float64)
    for ra in range(K):
        if 0 <= ra + dr < K:
            w += ((y - ra >= 0) & (y - ra < HP))
    return w


def _border_weights():
    """bw[p, (pass*NS+s)*4 + bi]: (wx_eff - wxc) at window cols {0,1,DW-2,DW-1}."""
    bw = np.zeros((128, 2 * NS * 4), np.float64)
    for pi, geom in enumerate(PASS_GEOM):
        DW = geom["DW"]
        for si, (dr, dc) in enumerate(SHIFTS):
            wxc = sum(1 for ca in range(K) if 0 <= ca + dc < K)
            for p in range(128):
                if pi == 0:
                    gx0, own_lo, own_hi = 0, 0, W
                else:
                    g = p // 64
                    gx0 = GROUP_X0[g]
                    own_lo, own_hi = (0, 192) if g == 0 else (192, W)
                for bi, j in enumerate((0, 1, DW - 2, DW - 1)):
                    x = gx0 + j
                    if own_lo <= x < own_hi and 0 <= x + dc < W and x < W:
                        wx = _wx_profile(dc, np.array([x]))[0]
                    else:
                        wx = 0.0
                    bw[p, (pi * NS + si) * 4 + bi] = wx - wxc
    return bw


_BW = None


def _host_inputs(logits, labels):
    in_maps = []
    for k in range(N_CORES):
        img, half = k // 2, k % 2
        g0 = half * ROWS_PER_CORE
        hi = min(H, g0 + LGROWS)
        lg = np.zeros((C, LGROWS, W), np.float32)
        lg[:, : hi - g0] = logits[img, :, g0:hi]
        lb = np.full((LGROWS, W), -1.0, np.float32)
        lb[: hi - g0] = labels[img, g0:hi].astype(np.float32)
        in_maps.append({
            "lg": lg.astype(ml_dtypes.bfloat16),
            "lb": lb.astype(ml_dtypes.bfloat16),
        })
    return in_maps


def _combine(accs_list, bcols_list):
    global _BW
    if _BW is None:
        _BW = _border_weights()
    total = 0.0
    for k in range(N_CORES):
        acc = accs_list[k].astype(np.float64)
        bc = bcols_list[k].astype(np.float64)
        g0 = (k % 2) * ROWS_PER_CORE
        for pi in range(2):
            p = np.arange(128)
            gy = g0 + p if pi == 0 else g0 + 128 + (p % 64)
            for si, (dr, dc) in enumerate(SHIFTS):
                mult = 1.0 if (dr, dc) == (0, 0) else 2.0
                wxc = float(sum(1 for ca in range(K) if 0 <= ca + dc < K))
                wy = _wy_profile(dr, gy)
                idx = pi * NS + si
                wb = _BW[:, idx * 4: idx * 4 + 4]
                full = acc[:, idx]
                border = (bc[:, idx * 4: idx * 4 + 4] * wb).sum(1)
                total += mult * np.sum(wy * (wxc * full + border))
    return total / (N * 81 * HP * WP)


_NC = None


def _build():
    global _NC
    if _NC is not None:
        return _NC
    from concourse import bacc, mybir
    import concourse.tile as tile

    f32 = mybir.dt.float32
    bf16 = mybir.dt.bfloat16
    Alu = mybir.AluOpType
    AF = mybir.ActivationFunctionType

    # All activations used here (Exp, Ln, Square, Sign) live together in the
    # "natural_log_exp_and_others" table set, but the table-load pass resolves
    # each function to the FIRST set containing it, which alternates sets and
    # reloads the ACT tables before nearly every activation.  Filter the other
    # sets' membership (indices untouched) so everything resolves to the one
    # shared set -> a single load.
    from concourse.hw_specs import get_activation_tables as _gat
    _keep = "natural_log_exp_and_others"
    _mine = {AF.Exp, AF.Ln, AF.Square, AF.Copy, AF.Sign, AF.Identity}

    def _gat_filtered(arch):
        t = _gat(arch)
        for name in t:
            if name != _keep:
                t[name] = t[name] - _mine
        return t

    bacc.get_activation_tables = _gat_filtered

    nc = bacc.Bacc("TRN2", target_bir_lowering=False, debug=False, num_devices=N_CORES)
    lg = nc.dram_tensor("lg", (C, LGROWS, W), bf16, kind="ExternalInput")
    lb = nc.dram_tensor("lb", (LGROWS, W), bf16, kind="ExternalInput")
    accs = nc.dram_tensor("accs", (128, COLS), f32, kind="ExternalOutput")
    bcols = nc.dram_tensor("bcols", (128, BCOLS), f32, kind="ExternalOutput")

    with tile.TileContext(nc) as tc:
        with tc.tile_pool(name="persist", bufs=1) as pool, \
             tc.tile_pool(name="work", bufs=2) as wpool, \
             tc.tile_pool(name="pbpool", bufs=1) as pbpool:
            accs_t = pool.tile([128, COLS], f32, name="accs_t")
            bcols_t = pool.tile([128, BCOLS], f32, name="bcols_t")
            nbias = pool.tile([128, 1], f32, name="nbias")
            nc.gpsimd.memset(nbias[:, :], -0.5)

            TW0 = PASS_GEOM[0]["TW"]

            def load_pass0_tiles():
                """T/Tod/L tiles for pass0, chunked DMAs in chain order."""
                row0, DW = 0, PASS_GEOM[0]["DW"]
                T, Tod, L = {}, {}, {}
                for dr in range(K):
                    t = pool.tile([128, C, TW0], bf16, tag=f"T{dr}", name=f"T{dr}")
                    to = pool.tile([128, C, TW0], bf16, tag=f"O{dr}", name=f"O{dr}")
                    lt = pool.tile([128, TW0], bf16, tag=f"L{dr}_0", name=f"L{dr}_0")
                    nc.gpsimd.memset(t[:, :, 0:2], 0)
                    nc.gpsimd.memset(t[:, :, 2 + DW:TW0], 0)
                    nc.gpsimd.memset(to[:, :, 0:1], 0)
                    nc.gpsimd.memset(to[:, :, 1 + DW:TW0], 0)
                    nc.gpsimd.memset(lt[:, 0:2], 0)
                    nc.gpsimd.memset(lt[:, 2 + DW:TW0], 0)
                    # per-queue DMA is ~22GB/s, so the first tiles are chunked
                    # across many queues to cut arrival latency
                    nchunk = 16 if dr == 0 else 8
                    bnds = [round(C * i / nchunk) for i in range(nchunk + 1)]
                    for c0, c1 in zip(bnds[:-1], bnds[1:]):
                        if c0 == c1:
                            continue
                        src = lg[c0:c1, row0 + dr:row0 + dr + 128, :].rearrange(
                            "c y x -> y c x")
                        nc.sync.dma_start(t[:, c0:c1, 2:2 + DW], src)
                    nc.sync.dma_start(lt[:, 2:2 + DW], lb[row0 + dr:row0 + dr + 128, :])
                    for c0, c1 in zip(bnds[:-1], bnds[1:]):
                        if c0 == c1:
                            continue
                        src = lg[c0:c1, row0 + dr:row0 + dr + 128, :].rearrange(
                            "c y x -> y c x")
                        nc.sync.dma_start(to[:, c0:c1, 1:1 + DW], src)
                    T[dr], Tod[dr], L[dr] = t, to, lt
                return T, Tod, L

            def load_pass1_fresh():
                """Fresh pass1 tiles (T0b + labels) - prefetchable."""
                row0, DW, TW = (PASS_GEOM[1][k] for k in ("row0", "DW", "TW"))
                t0b = pool.tile([128, C, TW], bf16, tag="T0b", name="T0b")
                nc.gpsimd.memset(t0b[:, :, 0:2], 0)
                nc.gpsimd.memset(t0b[:, :, 2 + DW:TW], 0)
                Ls = {}
                for dr in range(K):
                    lt = pool.tile([128, TW], bf16, tag=f"L{dr}_1", name=f"L{dr}_1")
                    nc.gpsimd.memset(lt[:, 0:2], 0)
                    nc.gpsimd.memset(lt[:, 2 + DW:TW], 0)
                    for g in range(2):
                        x0 = GROUP_X0[g]
                        nc.sync.dma_start(
                            lt[64 * g:64 * g + 64, 2:2 + DW],
                            lb[row0 + dr:row0 + dr + 64, x0:x0 + DW])
                    Ls[dr] = lt
                for g in range(2):
                    x0 = GROUP_X0[g]
                    for c0, c1 in ((0, 5), (5, 10), (10, 14), (14, C)):
                        src = lg[c0:c1, row0:row0 + 64, x0:x0 + DW
                                 ].rearrange("c y x -> y c x")
                        nc.sync.dma_start(
                            t0b[64 * g:64 * g + 64, c0:c1, 2:2 + DW], src)
                return t0b, Ls

            def load_pass1_reuse(tile_, dr, odd):
                """Overwrite a pass0 tile with its pass1 rows (after its
                pass0 readers; Tile framework tracks the WAR)."""
                row0, DW = PASS_GEOM[1]["row0"], PASS_GEOM[1]["DW"]
                off = 1 if odd else 2
                # only cols up to o1_max+DW ~ 198 are ever read in pass1
                nc.gpsimd.memset(tile_[:, :, off + DW:off + DW + 8], 0)
                for g in range(2):
                    x0 = GROUP_X0[g]
                    for c0, c1 in ((0, 10), (10, C)):
                        src = lg[c0:c1, row0 + dr:row0 + dr + 64, x0:x0 + DW
                                 ].rearrange("c y x -> y c x")
                        nc.sync.dma_start(
                            tile_[64 * g:64 * g + 64, c0:c1, off:off + DW], src)

            def emit_chain(ch, pi, DW, t0, t1, L, csplit=False):
                """One (dr, parity) chain: products -> tree -> sign -> softplus."""
                G = len(ch["dcs"])
                dr, kind = ch["dr"], ch["kind"]
                nm = f"{pi}_{dr}{kind}"
                pb = pbpool.tile([128, GMAX, 18, W], bf16, tag="pb", name=f"pb_{nm}")
                pc = wpool.tile([128, GMAX, 10, W], bf16, tag="pc", name=f"pc_{nm}")
                corr = wpool.tile([128, GMAX, W], bf16, tag="corr", name=f"corr_{nm}")
                u = wpool.tile([128, GMAX, W], bf16, tag="u", name=f"u_{nm}")
                if kind != 'sq':
                    dw = wpool.tile([128, GMAX, W], bf16, tag="dw", name=f"dw_{nm}")
                    sh = wpool.tile([128, GMAX, W], bf16, tag="sh", name=f"sh_{nm}")

                csp = ((0, 5), (5, 10), (10, 14), (14, 18)) if csplit else ((0, 18),)
                if kind == 'sq':
                    for c0, c1 in csp:
                        nc.scalar.activation(pb[:, 0, c0:c1, 0:DW],
                                             t0[:, c0:c1, 2:2 + DW], AF.Square)
                    nc.scalar.activation(pc[:, 0, 9, 0:DW],
                                         t0[:, 18, 2:2 + DW], AF.Square)
                else:
                    for g, dc in enumerate(ch["dcs"]):
                        o1 = (1 + dc) if kind == 'od' else (2 + dc)
                        for c0, c1 in csp:
                            nc.vector.tensor_tensor(
                                pb[:, g, c0:c1, 0:DW], t0[:, c0:c1, 2:2 + DW],
                                t1[:, c0:c1, o1:o1 + DW], Alu.mult)
                        nc.vector.tensor_tensor(
                            pc[:, g, 9, 0:DW], t0[:, 18, 2:2 + DW],
                            t1[:, 18, o1:o1 + DW], Alu.mult)

                # big tree levels on DVE (2x), low levels on GpSimd
                nc.vector.tensor_tensor(pc[:, 0:G, 0:9, 0:DW], pb[:, 0:G, 0:9, 0:DW],
                                        pb[:, 0:G, 9:18, 0:DW], Alu.add)
                nc.vector.tensor_tensor(pc[:, 0:G, 0:4, 0:DW], pc[:, 0:G, 0:4, 0:DW],
                                        pc[:, 0:G, 4:8, 0:DW], Alu.add)
                nc.gpsimd.tensor_tensor(pc[:, 0:G, 0:2, 0:DW], pc[:, 0:G, 0:2, 0:DW],
                                        pc[:, 0:G, 2:4, 0:DW], Alu.add)
                nc.gpsimd.tensor_tensor(pc[:, 0:G, 0, 0:DW], pc[:, 0:G, 0, 0:DW],
                                        pc[:, 0:G, 1, 0:DW], Alu.add)
                nc.gpsimd.tensor_tensor(pc[:, 0:G, 0, 0:DW], pc[:, 0:G, 0, 0:DW],
                                        pc[:, 0:G, 8, 0:DW], Alu.add)
                nc.gpsimd.tensor_tensor(corr[:, 0:G, 0:DW], pc[:, 0:G, 0, 0:DW],
                                        pc[:, 0:G, 9, 0:DW], Alu.add)

                if kind == 'sq':
                    # E == 1 everywhere: w = -corr, fold into Exp's scale
                    nc.scalar.activation(u[:, 0, 0:DW], corr[:, 0, 0:DW], AF.Exp,
                                         scale=-1.0)
                else:
                    # sh = 1-2E via label diff: sign(d^2 - 0.5)
                    for g, dc in enumerate(ch["dcs"]):
                        nc.gpsimd.tensor_tensor(
                            dw[:, g, 0:DW], L[0][:, 2:2 + DW],
                            L[dr][:, 2 + dc:2 + dc + DW], Alu.subtract)
                    nc.scalar.activation(sh[:, 0:G, 0:DW], dw[:, 0:G, 0:DW],
                                         AF.Square)
                    nc.scalar.activation(sh[:, 0:G, 0:DW], sh[:, 0:G, 0:DW],
                                         AF.Sign, bias=nbias[:, 0:1])
                    nc.gpsimd.tensor_tensor(dw[:, 0:G, 0:DW], sh[:, 0:G, 0:DW],
                                            corr[:, 0:G, 0:DW], Alu.mult)
                    nc.scalar.activation(u[:, 0:G, 0:DW], dw[:, 0:G, 0:DW], AF.Exp)

                for g in range(G):
                    idx = pi * NS + ch["sis"][g]
                    nc.scalar.activation(u[:, g, 0:DW], u[:, g, 0:DW], AF.Ln,
                                         bias=1.0, accum_out=accs_t[:, idx:idx + 1])
                    nc.gpsimd.tensor_copy(bcols_t[:, idx * 4:idx * 4 + 2],
                                          u[:, g, 0:2])
                    nc.gpsimd.tensor_copy(bcols_t[:, idx * 4 + 2:idx * 4 + 4],
                                          u[:, g, DW - 2:DW])

            # ---- pass 0 ----
            T, Tod, L0s = load_pass0_tiles()
            DW0 = PASS_GEOM[0]["DW"]

            for i, ch in enumerate(CHAINS):
                t1 = (Tod if ch["kind"] == 'od' else T)[ch["dr"]]
                emit_chain(ch, 0, DW0, T[0], t1, L0s, csplit=(i < 2))
                # prefetch pass1 data as soon as its pass0 buffer is free
                if ch["dr"] == 0 and ch["kind"] == 'od':
                    load_pass1_reuse(Tod[0], 0, odd=True)
                    t0b, L1s = load_pass1_fresh()
                elif ch["kind"] == 'ev' and ch["dr"] in (1, 2):
                    load_pass1_reuse(T[ch["dr"]], ch["dr"], odd=False)
                elif ch["kind"] == 'od' and ch["dr"] in (1, 2):
                    load_pass1_reuse(Tod[ch["dr"]], ch["dr"], odd=True)

            nc.sync.dma_start(accs[:, 0:NS], accs_t[:, 0:NS])
            nc.sync.dma_start(bcols[:, 0:NS * 4], bcols_t[:, 0:NS * 4])

            # ---- pass 1 ----
            DW1 = PASS_GEOM[1]["DW"]
            for ch in CHAINS:
                t0 = t0b
                t1 = t0b if (ch["dr"] == 0 and ch["kind"] != 'od') else \
                    (Tod if ch["kind"] == 'od' else T)[ch["dr"]]
                emit_chain(ch, 1, DW1, t0, t1, L1s, csplit=False)

            nc.sync.dma_start(accs[:, NS:2 * NS], accs_t[:, NS:2 * NS])
            nc.sync.dma_start(bcols[:, NS * 4:2 * NS * 4],
                              bcols_t[:, NS * 4:2 * NS * 4])

    nc.finalize()
    _NC = nc
    return nc


def kernel(logits, labels):
    nc = _build()
    in_maps = _host_inputs(np.asarray(logits, np.float32), np.asarray(labels))
    from concourse.bass_utils import run_bass_kernel_spmd
    res = run_bass_kernel_spmd(nc, in_maps, core_ids=list(range(N_CORES)))
    accs_list = [res.results[k]["accs"] for k in range(N_CORES)]
    bcols_list = [res.results[k]["bcols"] for k in range(N_CORES)]
    return np.array(_combine(accs_list, bcols_list), np.float32)


# revision 7
# speedup vs baseline: 1.1423x; 1.1423x over previous
"""AffinityLoss on 8 TRN2 NeuronCores (Bass/Tile).

Math: 3x3-unfold affinity loss = mean BCE-with-logits between per-pixel 9x9
channel Gram matrices and label-equality maps. Reformulated over 13 canonical
relative shifts s=(dr,dc) with integer edge-weight profiles wy/wx:

    loss_sum = sum_s mult_s * sum_{y,x} wy_s(y) wx_s(x) * ln(1 + exp(w_s(y,x)))
    w_s = (1 - 2*E_s) * Corr_s,   Corr_s(y,x) = sum_c A[c,y,x] A[c,y+dr,x+dc],
    E_s = [labels equal across the shift]

(BCE-with-logits == softplus((1-2E)*x); max |Corr| ~ 66 so exp is safe.)

Sharding: data-parallel, core k owns image k//2, rows [192*(k%2), +192).
Per core: pass0 = 128 rows full width; pass1 = 64 rows x 2 x-half groups
stacked on partitions.

Shifts are processed in CHAINS grouped by (dr, dc-parity) so the tree /
sign / exp stages run as single wide instructions over G=1-6 shifts (4D
tiles). Work split by measured engine rates (DVE 2x bf16 ~0.52ns/elem,
GpSimd ~3.4ns/elem):
  - DVE: bf16 products [G,19,DW] -> pb, tree L1 (pc[0:9]=pb[0:9]+pb[9:18]),
    L2 (pc[0:4]+=pc[4:8]), L5' (pc[8]+=pb[18]; frees pb on-engine).
  - GpSimd: low tree L3 (pc[0:2]+=pc[2:4]), L4 (pc[0]+=pc[1]),
    L6 (corr=pc[0]+pc[8]), and wt = sh*corr.
  - ScalarE: s=(0,0) products as Square, Exp, Ln(bias=1, accum_out)
    per-row loss sums, border-column copies.
The +-1 sign maps sh = 1-2E come precomputed from the host ("shm" input,
one [128,W] slab per (pass,shift)), like the other host-side label work.
wx border deviations live within 2 cols of the edges; those columns are
exported exactly and corrected on host in f64 together with the wy
weighting and the cross-core reduction.
"""

import os
import sys

import numpy as np

for _p in ("/root/.axon_site", "/root/.axon_site/_ro/trn_rl_repo",
           "/root/.axon_site/_ro/pypackages"):
    if os.path.isdir(_p) and _p not in sys.path:
        sys.path.append(_p)

import ml_dtypes  # noqa: E402

N, C, H, W = 4, 19, 384, 384
K = 3
HP = WP = H - K + 1  # 382
N_CORES = 8
ROWS_PER_CORE = 192
SHIFTS = [(0, 0), (0, 1), (0, 2)] + [(dr, dc) for dr in (1, 2) for dc in (-2, -1, 0, 1, 2)]
NS = len(SHIFTS)  # 13
PASS_GEOM = [
    dict(row0=0, DW=W, TW=W + 8),      # tiles [128, C, 392], data cols 2..385
    dict(row0=128, DW=194, TW=200),    # 64 rows x 2 x-half groups
]
GROUP_X0 = [0, 190]   # pass1 group g covers x in [190g, 190g+194)
COLS = 2 * NS        # one ln-accum column per (pass, shift)
BCOLS = 2 * NS * 4   # 4 exported border cols per (pass, shift)
LGROWS = 196

# chains: shifts sharing dc-alignment parity, each a list of parts
# (dr, odd?, dcs, sis); 'sq' = s=(0,0) square path.
CHAINS_P0 = [
    dict(kind='ev', parts=[dict(dr=0, odd=False, dcs=[2], sis=[2])]),
    dict(kind='sq', parts=[dict(dr=0, odd=False, dcs=[0], sis=[0])]),
    dict(kind='od', parts=[dict(dr=0, odd=True, dcs=[1], sis=[1])]),
    dict(kind='ev', parts=[dict(dr=1, odd=False, dcs=[-2, 0, 2], sis=[3, 5, 7])]),
    dict(kind='od', parts=[dict(dr=1, odd=True, dcs=[-1, 1], sis=[4, 6])]),
    dict(kind='ev', parts=[dict(dr=2, odd=False, dcs=[-2, 0, 2], sis=[8, 10, 12])]),
    dict(kind='od', parts=[dict(dr=2, odd=True, dcs=[-1, 1], sis=[9, 11])]),
]
CHAINS_P1 = [
    dict(kind='ev', parts=[dict(dr=0, odd=False, dcs=[2], sis=[2])]),
    dict(kind='sq', parts=[dict(dr=0, odd=False, dcs=[0], sis=[0])]),
    dict(kind='ev', parts=[dict(dr=1, odd=False, dcs=[-2, 0, 2], sis=[3, 5, 7]),
                           dict(dr=2, odd=False, dcs=[-2, 0, 2], sis=[8, 10, 12])]),
    dict(kind='od', parts=[dict(dr=0, odd=True, dcs=[1], sis=[1]),
                           dict(dr=1, odd=True, dcs=[-1, 1], sis=[4, 6]),
                           dict(dr=2, odd=True, dcs=[-1, 1], sis=[9, 11])]),
]
GMAX = 6

# host-precomputed sign-map slots, in kernel emission order
SLOTS = []
for _pi, _chs in ((0, CHAINS_P0), (1, CHAINS_P1)):
    for _ch in _chs:
        if _ch['kind'] == 'sq':
            continue
        for _pt in _ch['parts']:
            for _dc in _pt['dcs']:
                SLOTS.append((_pi, _pt['dr'], _dc))
NSLOTS = len(SLOTS)  # 24


def _wx_profile(dc, x):
    w = np.zeros_like(x, dtype=np.float64)
    for ca in range(K):
        if 0 <= ca + dc < K:
            w += ((x - ca >= 0) & (x - ca < WP))
    return w


def _wy_profile(dr, y):
    w = np.zeros_like(y, dtype=np.float64)
    for ra in range(K):
        if 0 <= ra + dr < K:
            w += ((y - ra >= 0) & (y - ra < HP))
    return w


def _border_weights():
    """bw[p, (pass*NS+s)*4 + bi]: (wx_eff - wxc) at window cols {0,1,DW-2,DW-1}."""
    bw = np.zeros((128, 2 * NS * 4), np.float64)
    for pi, geom in enumerate(PASS_GEOM):
        DW = geom["DW"]
        for si, (dr, dc) in enumerate(SHIFTS):
            wxc = sum(1 for ca in range(K) if 0 <= ca + dc < K)
            for p in range(128):
                if pi == 0:
                    gx0, own_lo, own_hi = 0, 0, W
                else:
                    g = p // 64
                    gx0 = GROUP_X0[g]
                    own_lo, own_hi = (0, 192) if g == 0 else (192, W)
                for bi, j in enumerate((0, 1, DW - 2, DW - 1)):
                    x = gx0 + j
                    if own_lo <= x < own_hi and 0 <= x + dc < W and x < W:
                        wx = _wx_profile(dc, np.array([x]))[0]
                    else:
                        wx = 0.0
                    bw[p, (pi * NS + si) * 4 + bi] = wx - wxc
    return bw


_BW = None


def _sign_maps(lbp):
    """shm[slot, p, x] = -1 where labels equal across the shift else +1.

    lbp: padded per-core labels [LGROWS, W] (float, -1 beyond the image).
    Out-of-range x+dc compares as not-equal (+1); those columns carry zero
    final weight (wx=0) or are border-corrected, so any finite value works.
    """
    shm = np.zeros((NSLOTS, 128, W), np.float32)
    for s, (pi, dr, dc) in enumerate(SLOTS):
        if pi == 0:
            rows = np.arange(128)
            a = lbp[rows]                      # [128, W]
            b = lbp[rows + dr]                 # [128, W]
            eq = np.zeros((128, W), bool)
            lo, hi = max(0, -dc), min(W, W - dc)
            eq[:, lo:hi] = a[:, lo:hi] == b[:, lo + dc:hi + dc]
            shm[s] = np.where(eq, -1.0, 1.0)
        else:
            for g in range(2):
                x0 = GROUP_X0[g]
                p = np.arange(64)
                rows = 128 + p
                a = lbp[rows][:, x0:x0 + 194]
                eq = np.zeros((64, 194), bool)
                xs = np.arange(x0, x0 + 194) + dc
                ok = (xs >= 0) & (xs < W)
                eq[:, ok] = a[:, ok] == lbp[rows + dr][:, xs[ok]]
                shm[s, 64 * g:64 * g + 64, 0:194] = np.where(eq, -1.0, 1.0)
    return shm


def _host_inputs(logits, labels):
    in_maps = []
    for k in range(N_CORES):
        img, half = k // 2, k % 2
        g0 = half * ROWS_PER_CORE
        hi = min(H, g0 + LGROWS)
        lg = np.zeros((C, LGROWS, W), np.float32)
        lg[:, : hi - g0] = logits[img, :, g0:hi]
        lbp = np.full((LGROWS, W), -1.0, np.float32)
        lbp[: hi - g0] = labels[img, g0:hi].astype(np.float32)
        in_maps.append({
            "lg": lg.astype(ml_dtypes.bfloat16),
            "shm": _sign_maps(lbp).astype(ml_dtypes.bfloat16),
        })
    return in_maps


def _combine(accs_list, bcols_list):
    global _BW
    if _BW is None:
        _BW = _border_weights()
    total = 0.0
    for k in range(N_CORES):
        acc = accs_list[k].astype(np.float64)
        bc = bcols_list[k].astype(np.float64)
        g0 = (k % 2) * ROWS_PER_CORE
        for pi in range(2):
            p = np.arange(128)
            gy = g0 + p if pi == 0 else g0 + 128 + (p % 64)
            for si, (dr, dc) in enumerate(SHIFTS):
                mult = 1.0 if (dr, dc) == (0, 0) else 2.0
                wxc = float(sum(1 for ca in range(K) if 0 <= ca + dc < K))
                wy = _wy_profile(dr, gy)
                idx = pi * NS + si
                wb = _BW[:, idx * 4: idx * 4 + 4]
                full = acc[:, idx]
                border = (bc[:, idx * 4: idx * 4 + 4] * wb).sum(1)
                total += mult * np.sum(wy * (wxc * full + border))
    return total / (N * 81 * HP * WP)


_NC = None


def _build():
    global _NC
    if _NC is not None:
        return _NC
    from concourse import bacc, mybir
    import concourse.tile as tile

    f32 = mybir.dt.float32
    bf16 = mybir.dt.bfloat16
    Alu = mybir.AluOpType
    AF = mybir.ActivationFunctionType

    # All activations used here (Exp, Ln, Square, Copy) live together in the
    # "natural_log_exp_and_others" table set, but the table-load pass resolves
    # each function to the FIRST set containing it, which alternates sets and
    # reloads the ACT tables before nearly every activation.  Filter the other
    # sets' membership (indices untouched) so everything resolves to the one
    # shared set -> a single load.
    from concourse.hw_specs import get_activation_tables as _gat
    _keep = "natural_log_exp_and_others"
    _mine = {AF.Exp, AF.Ln, AF.Square, AF.Copy, AF.Identity}

    def _gat_filtered(arch):
        t = _gat(arch)
        for name in t:
            if name != _keep:
                t[name] = t[name] - _mine
        return t

    bacc.get_activation_tables = _gat_filtered

    nc = bacc.Bacc("TRN2", target_bir_lowering=False, debug=False, num_devices=N_CORES)
    lg = nc.dram_tensor("lg", (C, LGROWS, W), bf16, kind="ExternalInput")
    shm = nc.dram_tensor("shm", (NSLOTS, 128, W), bf16, kind="ExternalInput")
    accs = nc.dram_tensor("accs", (128, COLS), f32, kind="ExternalOutput")
    bcols = nc.dram_tensor("bcols", (128, BCOLS), f32, kind="ExternalOutput")

    with tile.TileContext(nc) as tc:
        with tc.tile_pool(name="persist", bufs=1) as pool, \
             tc.tile_pool(name="work", bufs=2) as wpool, \
             tc.tile_pool(name="pbpool", bufs=1) as pbpool:
            accs_t = pool.tile([128, COLS], f32, name="accs_t")
            bcols_t = pool.tile([128, BCOLS], f32, name="bcols_t")

            TW0 = PASS_GEOM[0]["TW"]

            def load_pass0_tiles():
                """T/Tod tiles for pass0, chunked DMAs in chain order."""
                row0, DW = 0, PASS_GEOM[0]["DW"]
                T, Tod = {}, {}
                for dr in range(K):
                    t = pool.tile([128, C, TW0], bf16, tag=f"T{dr}", name=f"T{dr}")
                    to = pool.tile([128, C, TW0], bf16, tag=f"O{dr}", name=f"O{dr}")
                    nc.gpsimd.memset(t[:, :, 0:2], 0)
                    nc.gpsimd.memset(t[:, :, 2 + DW:TW0], 0)
                    nc.gpsimd.memset(to[:, :, 0:1], 0)
                    nc.gpsimd.memset(to[:, :, 1 + DW:TW0], 0)
                    # per-queue DMA is ~22GB/s, so the first tiles are chunked
                    # across many queues to cut arrival latency
                    nchunk = 16 if dr == 0 else 8
                    bnds = [round(C * i / nchunk) for i in range(nchunk + 1)]
                    for c0, c1 in zip(bnds[:-1], bnds[1:]):
                        if c0 == c1:
                            continue
                        src = lg[c0:c1, row0 + dr:row0 + dr + 128, :].rearrange(
                            "c y x -> y c x")
                        nc.sync.dma_start(t[:, c0:c1, 2:2 + DW], src)
                    for c0, c1 in zip(bnds[:-1], bnds[1:]):
                        if c0 == c1:
                            continue
                        src = lg[c0:c1, row0 + dr:row0 + dr + 128, :].rearrange(
                            "c y x -> y c x")
                        nc.sync.dma_start(to[:, c0:c1, 1:1 + DW], src)
                    T[dr], Tod[dr] = t, to
                return T, Tod

            def load_pass1_fresh():
                """Fresh pass1 T0 tile - prefetched during pass0."""
                row0, DW, TW = (PASS_GEOM[1][k] for k in ("row0", "DW", "TW"))
                t0b = pool.tile([128, C, TW], bf16, tag="T0b", name="T0b")
                nc.gpsimd.memset(t0b[:, :, 0:2], 0)
                nc.gpsimd.memset(t0b[:, :, 2 + DW:TW], 0)
                for g in range(2):
                    x0 = GROUP_X0[g]
                    for c0, c1 in ((0, 5), (5, 10), (10, 14), (14, C)):
                        src = lg[c0:c1, row0:row0 + 64, x0:x0 + DW
                                 ].rearrange("c y x -> y c x")
                        nc.sync.dma_start(
                            t0b[64 * g:64 * g + 64, c0:c1, 2:2 + DW], src)
                return t0b

            def load_pass1_reuse(tile_, dr, odd):
                """Overwrite a pass0 tile with its pass1 rows (after its
                pass0 readers; Tile framework tracks the WAR)."""
                row0, DW = PASS_GEOM[1]["row0"], PASS_GEOM[1]["DW"]
                off = 1 if odd else 2
                # only cols up to o1_max+DW ~ 198 are ever read in pass1
                nc.gpsimd.memset(tile_[:, :, off + DW:off + DW + 8], 0)
                for g in range(2):
                    x0 = GROUP_X0[g]
                    for c0, c1 in ((0, 10), (10, C)):
                        src = lg[c0:c1, row0 + dr:row0 + dr + 64, x0:x0 + DW
                                 ].rearrange("c y x -> y c x")
                        nc.sync.dma_start(
                            tile_[64 * g:64 * g + 64, c0:c1, off:off + DW], src)

            slot_ctr = [0]

            def emit_chain(ch, pi, DW, t0, T, Tod, csplit=False):
                """One chain: products -> tree -> sign-apply -> softplus."""
                G = sum(len(pt["dcs"]) for pt in ch["parts"])
                kind = ch["kind"]
                nm = f"{pi}_{ch['parts'][0]['dr']}{kind}"
                # tags size to the max over calls: pass0 uses [3, .., W],
                # pass1 [6, .., 200] - nearly the same bytes
                GP, WT = (3, W) if pi == 0 else (GMAX, 200)
                pb = pbpool.tile([128, GP, 19, WT], bf16, tag="pb", name=f"pb_{nm}")
                pc = wpool.tile([128, GP, 9, WT], bf16, tag="pc", name=f"pc_{nm}")
                corr = wpool.tile([128, GP, WT], bf16, tag="corr", name=f"corr_{nm}")
                u = wpool.tile([128, GP, WT], bf16, tag="u", name=f"u_{nm}")
                if kind != 'sq':
                    sh = wpool.tile([128, GP, WT], bf16, tag="sh", name=f"sh_{nm}")
                    s0 = slot_ctr[0]
                    nc.sync.dma_start(
                        sh[:, 0:G, 0:DW],
                        shm[s0:s0 + G, :, 0:DW].rearrange("s p x -> p s x"))
                    slot_ctr[0] += G

                csp = ((0, 5), (5, 10), (10, 14), (14, 19)) if csplit \
                    else ((0, 19),)
                if kind == 'sq':
                    for c0, c1 in csp:
                        nc.scalar.activation(pb[:, 0, c0:c1, 0:DW],
                                             t0[:, c0:c1, 2:2 + DW], AF.Square)
                else:
                    g = 0
                    for pt in ch["parts"]:
                        t1 = (Tod if pt["odd"] else T)[pt["dr"]]
                        for dc in pt["dcs"]:
                            o1 = (1 + dc) if pt["odd"] else (2 + dc)
                            for c0, c1 in csp:
                                nc.vector.tensor_tensor(
                                    pb[:, g, c0:c1, 0:DW], t0[:, c0:c1, 2:2 + DW],
                                    t1[:, c0:c1, o1:o1 + DW], Alu.mult)
                            g += 1

                # big tree levels on DVE (2x); low levels + sign-mult on GpSimd
                nc.vector.tensor_tensor(pc[:, 0:G, 0:9, 0:DW], pb[:, 0:G, 0:9, 0:DW],
                                        pb[:, 0:G, 9:18, 0:DW], Alu.add)
                nc.vector.tensor_tensor(pc[:, 0:G, 0:4, 0:DW], pc[:, 0:G, 0:4, 0:DW],
                                        pc[:, 0:G, 4:8, 0:DW], Alu.add)
                nc.vector.tensor_tensor(pc[:, 0:G, 8:9, 0:DW], pc[:, 0:G, 8:9, 0:DW],
                                        pb[:, 0:G, 18:19, 0:DW], Alu.add)
                nc.gpsimd.tensor_tensor(pc[:, 0:G, 0:2, 0:DW], pc[:, 0:G, 0:2, 0:DW],
                                        pc[:, 0:G, 2:4, 0:DW], Alu.add)
                nc.gpsimd.tensor_tensor(pc[:, 0:G, 0, 0:DW], pc[:, 0:G, 0, 0:DW],
                                        pc[:, 0:G, 1, 0:DW], Alu.add)
                nc.gpsimd.tensor_tensor(corr[:, 0:G, 0:DW], pc[:, 0:G, 0, 0:DW],
                                        pc[:, 0:G, 8, 0:DW], Alu.add)

                if kind == 'sq':
                    # E == 1 everywhere: w = -corr, fold into Exp's scale
                    nc.scalar.activation(u[:, 0, 0:DW], corr[:, 0, 0:DW], AF.Exp,
                                         scale=-1.0)
                else:
                    nc.gpsimd.tensor_tensor(u[:, 0:G, 0:DW], sh[:, 0:G, 0:DW],
                                            corr[:, 0:G, 0:DW], Alu.mult)
                    nc.scalar.activation(u[:, 0:G, 0:DW], u[:, 0:G, 0:DW], AF.Exp)

                g = 0
                for pt in ch["parts"]:
                    for si in pt["sis"]:
                        idx = pi * NS + si
                        nc.scalar.activation(u[:, g, 0:DW], u[:, g, 0:DW], AF.Ln,
                                             bias=1.0,
                                             accum_out=accs_t[:, idx:idx + 1])
                        nc.scalar.copy(bcols_t[:, idx * 4:idx * 4 + 2],
                                       u[:, g, 0:2])
                        nc.scalar.copy(bcols_t[:, idx * 4 + 2:idx * 4 + 4],
                                       u[:, g, DW - 2:DW])
                        g += 1

            # ---- pass 0 ----
            T, Tod = load_pass0_tiles()
            DW0 = PASS_GEOM[0]["DW"]

            for i, ch in enumerate(CHAINS_P0):
                emit_chain(ch, 0, DW0, T[0], T, Tod, csplit=(i < 2))
                # prefetch pass1 data as soon as its pass0 buffer is free
                pt = ch["parts"][0]
                if pt["dr"] == 0 and ch["kind"] == 'od':
                    load_pass1_reuse(Tod[0], 0, odd=True)
                    t0b = load_pass1_fresh()
                elif ch["kind"] == 'ev' and pt["dr"] in (1, 2):
                    load_pass1_reuse(T[pt["dr"]], pt["dr"], odd=False)
                elif ch["kind"] == 'od' and pt["dr"] in (1, 2):
                    load_pass1_reuse(Tod[pt["dr"]], pt["dr"], odd=True)

            nc.sync.dma_start(accs[:, 0:NS], accs_t[:, 0:NS])
            nc.sync.dma_start(bcols[:, 0:NS * 4], bcols_t[:, 0:NS * 4])

            # ---- pass 1 ----
            DW1 = PASS_GEOM[1]["DW"]
            Tp1 = {0: t0b, 1: T[1], 2: T[2]}
            for ch in CHAINS_P1:
                emit_chain(ch, 1, DW1, t0b, Tp1, Tod, csplit=False)

            nc.sync.dma_start(accs[:, NS:2 * NS], accs_t[:, NS:2 * NS])
            nc.sync.dma_start(bcols[:, NS * 4:2 * NS * 4],
                              bcols_t[:, NS * 4:2 * NS * 4])

    nc.finalize()
    _NC = nc
    return nc


def kernel(logits, labels):
    nc = _build()
    in_maps = _host_inputs(np.asarray(logits, np.float32), np.asarray(labels))
    from concourse.bass_utils import run_bass_kernel_spmd
    res = run_bass_kernel_spmd(nc, in_maps, core_ids=list(range(N_CORES)))
    accs_list = [res.results[k]["accs"] for k in range(N_CORES)]
    bcols_list = [res.results[k]["bcols"] for k in range(N_CORES)]
    return np.array(_combine(accs_list, bcols_list), np.float32)


# revision 8
# speedup vs baseline: 1.3800x; 1.2081x over previous
"""AffinityLoss on 8 TRN2 NeuronCores (Bass/Tile).

Math: 3x3-unfold affinity loss = mean BCE-with-logits between per-pixel 9x9
channel Gram matrices and label-equality maps. Reformulated over 13 canonical
relative shifts s=(dr,dc) with integer edge-weight profiles wy/wx:

    loss_sum = sum_s mult_s * sum_{y,x} wy_s(y) wx_s(x) * ln(1 + exp(w_s(y,x)))
    w_s = (1 - 2*E_s) * Corr_s,   Corr_s(y,x) = sum_c A[c,y,x] A[c,y+dr,x+dc],
    E_s = [labels equal across the shift]

(BCE-with-logits == softplus((1-2E)*x); max |Corr| ~ 66 so exp is safe.)

Sharding: data-parallel, core k owns image k//2, rows [192*(k%2), +192).
Per core: pass0 = 128 rows full width; pass1 = 64 rows x 2 x-half groups
stacked on partitions.

Shifts are processed in CHAINS grouped by (dr, dc-parity) so the tree /
sign / exp stages run as single wide instructions over G=1-6 shifts (4D
tiles). Work split by measured engine rates (DVE 2x bf16 ~0.52ns/elem,
GpSimd ~3.4ns/elem):
  - DVE: bf16 products [G,19,DW] -> pb, tree L1 (pc[0:9]=pb[0:9]+pb[9:18]),
    L2 (pc[0:4]+=pc[4:8]), L5' (pc[8]+=pb[18]; frees pb on-engine).
  - GpSimd: low tree L3 (pc[0:2]+=pc[2:4]), L4 (pc[0]+=pc[1]),
    L6 (corr=pc[0]+pc[8]), and wt = sh*corr.
  - ScalarE: s=(0,0) products as Square, Exp, Ln(bias=1, accum_out)
    per-row loss sums, border-column copies.
The +-1 sign maps sh = 1-2E come precomputed from the host ("shm" input,
one [128,W] slab per (pass,shift)), like the other host-side label work.
wx border deviations live within 2 cols of the edges; those columns are
exported exactly and corrected on host in f64 together with the wy
weighting and the cross-core reduction.
"""

import os
import sys

import numpy as np

for _p in ("/root/.axon_site", "/root/.axon_site/_ro/trn_rl_repo",
           "/root/.axon_site/_ro/pypackages"):
    if os.path.isdir(_p) and _p not in sys.path:
        sys.path.append(_p)

import ml_dtypes  # noqa: E402

N, C, H, W = 4, 19, 384, 384
K = 3
HP = WP = H - K + 1  # 382
N_CORES = 8
ROWS_PER_CORE = 192
SHIFTS = [(0, 0), (0, 1), (0, 2)] + [(dr, dc) for dr in (1, 2) for dc in (-2, -1, 0, 1, 2)]
NS = len(SHIFTS)  # 13
PASS_GEOM = [
    dict(row0=0, DW=W, TW=W + 8),      # tiles [128, C, 392], data cols 2..385
    dict(row0=128, DW=194, TW=200),    # 64 rows x 2 x-half groups
]
GROUP_X0 = [0, 190]   # pass1 group g covers x in [190g, 190g+194)
COLS = 2 * NS        # one ln-accum column per (pass, shift)
BCOLS = 2 * NS * 4   # 4 exported border cols per (pass, shift)
LGROWS = 196

# chains: shifts sharing dc-alignment parity, each a list of parts
# (dr, odd?, dcs, sis); 'sq' = s=(0,0) square path.
CHAINS_P0 = [
    dict(kind='ev', parts=[dict(dr=0, odd=False, dcs=[2], sis=[2])]),
    dict(kind='sq', parts=[dict(dr=0, odd=False, dcs=[0], sis=[0])]),
    dict(kind='od', parts=[dict(dr=0, odd=True, dcs=[1], sis=[1])]),
    dict(kind='ev', parts=[dict(dr=1, odd=False, dcs=[-2, 0, 2], sis=[3, 5, 7])]),
    dict(kind='od', parts=[dict(dr=1, odd=True, dcs=[-1, 1], sis=[4, 6])]),
    dict(kind='ev', parts=[dict(dr=2, odd=False, dcs=[-2, 0, 2], sis=[8, 10, 12])]),
    dict(kind='od', parts=[dict(dr=2, odd=True, dcs=[-1, 1], sis=[9, 11])]),
]
CHAINS_P1 = [
    dict(kind='ev', parts=[dict(dr=1, odd=False, dcs=[-2, 0, 2], sis=[3, 5, 7]),
                           dict(dr=2, odd=False, dcs=[-2, 0, 2], sis=[8, 10, 12])]),
    dict(kind='od', parts=[dict(dr=0, odd=True, dcs=[1], sis=[1]),
                           dict(dr=1, odd=True, dcs=[-1, 1], sis=[4, 6]),
                           dict(dr=2, odd=True, dcs=[-1, 1], sis=[9, 11])]),
    dict(kind='ev', parts=[dict(dr=0, odd=False, dcs=[2], sis=[2])]),
    dict(kind='sq', parts=[dict(dr=0, odd=False, dcs=[0], sis=[0])]),
]
GMAX = 6

# host-precomputed sign-map slots, in kernel emission order
SLOTS = []
for _pi, _chs in ((0, CHAINS_P0), (1, CHAINS_P1)):
    for _ch in _chs:
        if _ch['kind'] == 'sq':
            continue
        for _pt in _ch['parts']:
            for _dc in _pt['dcs']:
                SLOTS.append((_pi, _pt['dr'], _dc))
NSLOTS = len(SLOTS)  # 24


def _wx_profile(dc, x):
    w = np.zeros_like(x, dtype=np.float64)
    for ca in range(K):
        if 0 <= ca + dc < K:
            w += ((x - ca >= 0) & (x - ca < WP))
    return w


def _wy_profile(dr, y):
    w = np.zeros_like(y, dtype=np.float64)
    for ra in range(K):
        if 0 <= ra + dr < K:
            w += ((y - ra >= 0) & (y - ra < HP))
    return w


def _border_weights():
    """bw[p, (pass*NS+s)*4 + bi]: (wx_eff - wxc) at window cols {0,1,DW-2,DW-1}."""
    bw = np.zeros((128, 2 * NS * 4), np.float64)
    for pi, geom in enumerate(PASS_GEOM):
        DW = geom["DW"]
        for si, (dr, dc) in enumerate(SHIFTS):
            wxc = sum(1 for ca in range(K) if 0 <= ca + dc < K)
            for p in range(128):
                if pi == 0:
                    gx0, own_lo, own_hi = 0, 0, W
                else:
                    g = p // 64
                    gx0 = GROUP_X0[g]
                    own_lo, own_hi = (0, 192) if g == 0 else (192, W)
                for bi, j in enumerate((0, 1, DW - 2, DW - 1)):
                    x = gx0 + j
                    if own_lo <= x < own_hi and 0 <= x + dc < W and x < W:
                        wx = _wx_profile(dc, np.array([x]))[0]
                    else:
                        wx = 0.0
                    bw[p, (pi * NS + si) * 4 + bi] = wx - wxc
    return bw


_BW = None


def _sign_maps(lbp):
    """shm[slot, p, x] = -1 where labels equal across the shift else +1.

    lbp: padded per-core labels [LGROWS, W] (float, -1 beyond the image).
    Out-of-range x+dc compares as not-equal (+1); those columns carry zero
    final weight (wx=0) or are border-corrected, so any finite value works.
    """
    shm = np.zeros((NSLOTS, 128, W), np.float32)
    for s, (pi, dr, dc) in enumerate(SLOTS):
        if pi == 0:
            rows = np.arange(128)
            a = lbp[rows]                      # [128, W]
            b = lbp[rows + dr]                 # [128, W]
            eq = np.zeros((128, W), bool)
            lo, hi = max(0, -dc), min(W, W - dc)
            eq[:, lo:hi] = a[:, lo:hi] == b[:, lo + dc:hi + dc]
            shm[s] = np.where(eq, -1.0, 1.0)
        else:
            for g in range(2):
                x0 = GROUP_X0[g]
                p = np.arange(64)
                rows = 128 + p
                a = lbp[rows][:, x0:x0 + 194]
                eq = np.zeros((64, 194), bool)
                xs = np.arange(x0, x0 + 194) + dc
                ok = (xs >= 0) & (xs < W)
                eq[:, ok] = a[:, ok] == lbp[rows + dr][:, xs[ok]]
                shm[s, 64 * g:64 * g + 64, 0:194] = np.where(eq, -1.0, 1.0)
    return shm


def _host_inputs(logits, labels):
    in_maps = []
    for k in range(N_CORES):
        img, half = k // 2, k % 2
        g0 = half * ROWS_PER_CORE
        hi = min(H, g0 + LGROWS)
        lg = np.zeros((C, LGROWS, W), np.float32)
        lg[:, : hi - g0] = logits[img, :, g0:hi]
        lbp = np.full((LGROWS, W), -1.0, np.float32)
        lbp[: hi - g0] = labels[img, g0:hi].astype(np.float32)
        in_maps.append({
            "lg": lg.astype(ml_dtypes.bfloat16),
            "shm": _sign_maps(lbp).astype(ml_dtypes.bfloat16),
        })
    return in_maps


def _combine(accs_list, bcols_list):
    global _BW
    if _BW is None:
        _BW = _border_weights()
    total = 0.0
    for k in range(N_CORES):
        acc = accs_list[k].astype(np.float64)
        bc = bcols_list[k].astype(np.float64)
        g0 = (k % 2) * ROWS_PER_CORE
        for pi in range(2):
            p = np.arange(128)
            gy = g0 + p if pi == 0 else g0 + 128 + (p % 64)
            for si, (dr, dc) in enumerate(SHIFTS):
                mult = 1.0 if (dr, dc) == (0, 0) else 2.0
                wxc = float(sum(1 for ca in range(K) if 0 <= ca + dc < K))
                wy = _wy_profile(dr, gy)
                idx = pi * NS + si
                wb = _BW[:, idx * 4: idx * 4 + 4]
                full = acc[:, idx]
                border = (bc[:, idx * 4: idx * 4 + 4] * wb).sum(1)
                total += mult * np.sum(wy * (wxc * full + border))
    return total / (N * 81 * HP * WP)


_NC = None


def _build():
    global _NC
    if _NC is not None:
        return _NC
    from concourse import bacc, mybir
    import concourse.tile as tile

    f32 = mybir.dt.float32
    bf16 = mybir.dt.bfloat16
    Alu = mybir.AluOpType
    AF = mybir.ActivationFunctionType

    # All activations used here (Exp, Ln, Square, Copy) live together in the
    # "natural_log_exp_and_others" table set, but the table-load pass resolves
    # each function to the FIRST set containing it, which alternates sets and
    # reloads the ACT tables before nearly every activation.  Filter the other
    # sets' membership (indices untouched) so everything resolves to the one
    # shared set -> a single load.
    from concourse.hw_specs import get_activation_tables as _gat
    _keep = "natural_log_exp_and_others"
    _mine = {AF.Exp, AF.Ln, AF.Square, AF.Copy, AF.Identity}

    def _gat_filtered(arch):
        t = _gat(arch)
        for name in t:
            if name != _keep:
                t[name] = t[name] - _mine
        return t

    bacc.get_activation_tables = _gat_filtered

    nc = bacc.Bacc("TRN2", target_bir_lowering=False, debug=False, num_devices=N_CORES)
    lg = nc.dram_tensor("lg", (C, LGROWS, W), bf16, kind="ExternalInput")
    shm = nc.dram_tensor("shm", (NSLOTS, 128, W), bf16, kind="ExternalInput")
    accs = nc.dram_tensor("accs", (128, COLS), f32, kind="ExternalOutput")
    bcols = nc.dram_tensor("bcols", (128, BCOLS), f32, kind="ExternalOutput")

    with tile.TileContext(nc) as tc:
        with tc.tile_pool(name="persist", bufs=1) as pool, \
             tc.tile_pool(name="work", bufs=2) as wpool, \
             tc.tile_pool(name="pbpool", bufs=1) as pbpool:
            accs_t = pool.tile([128, COLS], f32, name="accs_t")
            bcols_t = pool.tile([128, BCOLS], f32, name="bcols_t")

            TW0 = PASS_GEOM[0]["TW"]

            def load_pass0_tiles():
                """T/Tod tiles for pass0, chunked DMAs in chain order."""
                row0, DW = 0, PASS_GEOM[0]["DW"]
                T, Tod = {}, {}
                for dr in range(K):
                    t = pool.tile([128, C, TW0], bf16, tag=f"T{dr}", name=f"T{dr}")
                    to = pool.tile([128, C, TW0], bf16, tag=f"O{dr}", name=f"O{dr}")
                    nc.gpsimd.memset(t[:, :, 0:2], 0)
                    nc.gpsimd.memset(t[:, :, 2 + DW:TW0], 0)
                    nc.gpsimd.memset(to[:, :, 0:1], 0)
                    nc.gpsimd.memset(to[:, :, 1 + DW:TW0], 0)
                    # per-queue DMA is ~22GB/s, so the first tiles are chunked
                    # across many queues to cut arrival latency
                    nchunk = 16 if dr == 0 else 8
                    bnds = [round(C * i / nchunk) for i in range(nchunk + 1)]
                    for c0, c1 in zip(bnds[:-1], bnds[1:]):
                        if c0 == c1:
                            continue
                        src = lg[c0:c1, row0 + dr:row0 + dr + 128, :].rearrange(
                            "c y x -> y c x")
                        nc.sync.dma_start(t[:, c0:c1, 2:2 + DW], src)
                    for c0, c1 in zip(bnds[:-1], bnds[1:]):
                        if c0 == c1:
                            continue
                        src = lg[c0:c1, row0 + dr:row0 + dr + 128, :].rearrange(
                            "c y x -> y c x")
                        nc.sync.dma_start(to[:, c0:c1, 1:1 + DW], src)
                    T[dr], Tod[dr] = t, to
                return T, Tod

            def load_pass1_fresh():
                """Fresh pass1 T0 tile - prefetched during pass0."""
                row0, DW, TW = (PASS_GEOM[1][k] for k in ("row0", "DW", "TW"))
                t0b = pool.tile([128, C, TW], bf16, tag="T0b", name="T0b")
                nc.gpsimd.memset(t0b[:, :, 0:2], 0)
                nc.gpsimd.memset(t0b[:, :, 2 + DW:TW], 0)
                for g in range(2):
                    x0 = GROUP_X0[g]
                    for c0, c1 in ((0, 5), (5, 10), (10, 14), (14, C)):
                        src = lg[c0:c1, row0:row0 + 64, x0:x0 + DW
                                 ].rearrange("c y x -> y c x")
                        nc.sync.dma_start(
                            t0b[64 * g:64 * g + 64, c0:c1, 2:2 + DW], src)
                return t0b

            def load_pass1_reuse(tile_, dr, odd):
                """Overwrite a pass0 tile with its pass1 rows (after its
                pass0 readers; Tile framework tracks the WAR)."""
                row0, DW = PASS_GEOM[1]["row0"], PASS_GEOM[1]["DW"]
                off = 1 if odd else 2
                # only cols up to o1_max+DW ~ 198 are ever read in pass1
                nc.gpsimd.memset(tile_[:, :, off + DW:off + DW + 8], 0)
                for g in range(2):
                    x0 = GROUP_X0[g]
                    for c0, c1 in ((0, 10), (10, C)):
                        src = lg[c0:c1, row0 + dr:row0 + dr + 64, x0:x0 + DW
                                 ].rearrange("c y x -> y c x")
                        nc.sync.dma_start(
                            tile_[64 * g:64 * g + 64, c0:c1, off:off + DW], src)

            slot_ctr = [0]

            def emit_chain(ch, pi, DW, t0, T, Tod, csplit=False):
                """One chain: products -> tree -> sign-apply -> softplus."""
                G = sum(len(pt["dcs"]) for pt in ch["parts"])
                kind = ch["kind"]
                nm = f"{pi}_{ch['parts'][0]['dr']}{kind}"
                # tags size to the max over calls: pass0 uses [3, .., W],
                # pass1 [6, .., 200] - nearly the same bytes
                GP, WT = (3, W) if pi == 0 else (GMAX, 200)
                pb = pbpool.tile([128, GP, 19, WT], bf16, tag="pb", name=f"pb_{nm}")
                pc = wpool.tile([128, GP, 9, WT], bf16, tag="pc", name=f"pc_{nm}")
                corr = wpool.tile([128, GP, WT], bf16, tag="corr", name=f"corr_{nm}")
                u = wpool.tile([128, GP, WT], bf16, tag="u", name=f"u_{nm}")
                if kind != 'sq':
                    sh = wpool.tile([128, GP, WT], bf16, tag="sh", name=f"sh_{nm}")
                    s0 = slot_ctr[0]
                    nc.sync.dma_start(
                        sh[:, 0:G, 0:DW],
                        shm[s0:s0 + G, :, 0:DW].rearrange("s p x -> p s x"))
                    slot_ctr[0] += G

                csp = ((0, 5), (5, 10), (10, 14), (14, 19)) if csplit \
                    else ((0, 19),)
                if kind == 'sq':
                    for c0, c1 in csp:
                        nc.scalar.activation(pb[:, 0, c0:c1, 0:DW],
                                             t0[:, c0:c1, 2:2 + DW], AF.Square)
                else:
                    g = 0
                    for pt in ch["parts"]:
                        t1 = (Tod if pt["odd"] else T)[pt["dr"]]
                        for dc in pt["dcs"]:
                            o1 = (1 + dc) if pt["odd"] else (2 + dc)
                            for c0, c1 in csp:
                                nc.vector.tensor_tensor(
                                    pb[:, g, c0:c1, 0:DW], t0[:, c0:c1, 2:2 + DW],
                                    t1[:, c0:c1, o1:o1 + DW], Alu.mult)
                            g += 1

                # big tree levels on DVE (2x); low levels + sign-mult on GpSimd
                nc.vector.tensor_tensor(pc[:, 0:G, 0:9, 0:DW], pb[:, 0:G, 0:9, 0:DW],
                                        pb[:, 0:G, 9:18, 0:DW], Alu.add)
                nc.vector.tensor_tensor(pc[:, 0:G, 0:4, 0:DW], pc[:, 0:G, 0:4, 0:DW],
                                        pc[:, 0:G, 4:8, 0:DW], Alu.add)
                nc.vector.tensor_tensor(pc[:, 0:G, 8:9, 0:DW], pc[:, 0:G, 8:9, 0:DW],
                                        pb[:, 0:G, 18:19, 0:DW], Alu.add)
                nc.vector.tensor_tensor(pc[:, 0:G, 0:2, 0:DW], pc[:, 0:G, 0:2, 0:DW],
                                        pc[:, 0:G, 2:4, 0:DW], Alu.add)
                nc.vector.tensor_tensor(pc[:, 0:G, 0, 0:DW], pc[:, 0:G, 0, 0:DW],
                                        pc[:, 0:G, 1, 0:DW], Alu.add)
                nc.vector.tensor_tensor(corr[:, 0:G, 0:DW], pc[:, 0:G, 0, 0:DW],
                                        pc[:, 0:G, 8, 0:DW], Alu.add)

                if kind == 'sq':
                    # E == 1 everywhere: w = -corr, fold into Exp's scale
                    nc.scalar.activation(u[:, 0, 0:DW], corr[:, 0, 0:DW], AF.Exp,
                                         scale=-1.0)
                else:
                    nc.vector.tensor_tensor(u[:, 0:G, 0:DW], sh[:, 0:G, 0:DW],
                                            corr[:, 0:G, 0:DW], Alu.mult)
                    nc.scalar.activation(u[:, 0:G, 0:DW], u[:, 0:G, 0:DW], AF.Exp)

                g = 0
                for pt in ch["parts"]:
                    for si in pt["sis"]:
                        idx = pi * NS + si
                        nc.scalar.activation(u[:, g, 0:DW], u[:, g, 0:DW], AF.Ln,
                                             bias=1.0,
                                             accum_out=accs_t[:, idx:idx + 1])
                        nc.scalar.copy(bcols_t[:, idx * 4:idx * 4 + 2],
                                       u[:, g, 0:2])
                        nc.scalar.copy(bcols_t[:, idx * 4 + 2:idx * 4 + 4],
                                       u[:, g, DW - 2:DW])
                        g += 1

            # ---- pass 0 ----
            T, Tod = load_pass0_tiles()
            DW0 = PASS_GEOM[0]["DW"]

            for i, ch in enumerate(CHAINS_P0):
                emit_chain(ch, 0, DW0, T[0], T, Tod, csplit=(i < 2))
                # prefetch pass1 data as soon as its pass0 buffer is free
                pt = ch["parts"][0]
                if pt["dr"] == 0 and ch["kind"] == 'od':
                    load_pass1_reuse(Tod[0], 0, odd=True)
                    t0b = load_pass1_fresh()
                elif ch["kind"] == 'ev' and pt["dr"] in (1, 2):
                    load_pass1_reuse(T[pt["dr"]], pt["dr"], odd=False)
                elif ch["kind"] == 'od' and pt["dr"] in (1, 2):
                    load_pass1_reuse(Tod[pt["dr"]], pt["dr"], odd=True)

            nc.sync.dma_start(accs[:, 0:NS], accs_t[:, 0:NS])
            nc.sync.dma_start(bcols[:, 0:NS * 4], bcols_t[:, 0:NS * 4])

            # ---- pass 1 ----
            DW1 = PASS_GEOM[1]["DW"]
            Tp1 = {0: t0b, 1: T[1], 2: T[2]}
            for ch in CHAINS_P1:
                emit_chain(ch, 1, DW1, t0b, Tp1, Tod, csplit=False)

            nc.sync.dma_start(accs[:, NS:2 * NS], accs_t[:, NS:2 * NS])
            nc.sync.dma_start(bcols[:, NS * 4:2 * NS * 4],
                              bcols_t[:, NS * 4:2 * NS * 4])

    nc.finalize()
    _NC = nc
    return nc


def kernel(logits, labels):
    nc = _build()
    in_maps = _host_inputs(np.asarray(logits, np.float32), np.asarray(labels))
    from concourse.bass_utils import run_bass_kernel_spmd
    res = run_bass_kernel_spmd(nc, in_maps, core_ids=list(range(N_CORES)))
    accs_list = [res.results[k]["accs"] for k in range(N_CORES)]
    bcols_list = [res.results[k]["bcols"] for k in range(N_CORES)]
    return np.array(_combine(accs_list, bcols_list), np.float32)
